# revision 19
# baseline (speedup 1.0000x reference)
"""CABlock (cross-attention block) Trainium2 Bass kernel.

Problem: b=8, c=64, h=w=48 (n=2304), CR=8.
  qk_i = Wqk_i @ x_i + bqk_i  (q = first 8 rows, k = last 8)
  attn_i = softmax_j(q_i^T k_i)            [n, n]
  o1 = (Wv1@x1 + bv1) @ attn2 * gamma + x1
  o2 = (Wv2@x2 + bv2) @ attn1 * beta  + x2

Sharding: data-parallel over batch, 1 batch element per NeuronCore (8 cores).

Per-core dataflow (channel-on-partition).  Tile deps are whole-tensor, so
everything the pipeline must overlap lives in SEPARATE tensors:
  - x arrives as five [128, 512] column-chunk tensors (x1 @ partitions 0:64,
    x2 @ 64:128), DMA'd per chunk so projections start ~1us in.
  - q/k are projected per 512-col chunk into per-(attn, chunk) SBUF tensors
    (fp32r, rows 0:8).  The projection matmuls borrow the psum_o banks as
    scratch (free until the first o-matmul, which is deferred to tile 2).
  - A-row (128 queries x 2304 keys) streams through two PSUM scratch
    tiles T2=[128,1024] (2 banks) and T1=[128,512] (1 bank) as chunks
    (1024, 512, 512, 256) -> only 4 ScalarE exp instructions per row
    (multi-bank PSUM reads are fine for ACT), amortizing the ~185ns/instr
    ACT overhead.  The T2/T1/T2/T1 alternation keeps PE a chunk ahead.
    A-matmul pieces align with the 512-col k chunks.  No max subtraction
    (logit range is far inside fp32 exp range).
  - Row sums skip accum_out (187ns/instr on the bottleneck ACT engine):
    DVE folds the bf16 E row 2304->1152->576->288 (tensor_tensor runs at
    2x for bf16) and tensor_reduces the last 288.
  - V^T tiles are computed inside the main loop (row (t,an) projects its
    own attn's tile through spare at1 columns 256:320), so no pre-phase.
  - 1/s (and gamma/beta) folded into the tiny [128, 64] V^T tiles.  E and
    V^T in bf16 -> o-matmuls run at full rate; the attention term is scaled
    by gamma/beta = 0.1, so bf16 rounding there is ~5e-4 relative to the
    residual-dominated output.
  - o1/o2 column-packed into one PSUM accumulator [128, 2304] (5 banks,
    padded to 2560 for zero-region alignment), accumulated over all 18
    i-tiles via start/stop.  The residual is an identity matmul (stop=True)
    appended to the accumulation; the PSUM->SBUF move runs on the
    then-idle ScalarE (Copy), chunk-interleaved with the output DMAs.
"""

import numpy as np

C = 64
CR = 8
H = W = 48
N = H * W            # 2304
B = 8
P = 128
IT = N // P          # 18 i-tiles
# 512-col chunks (each within one 2KB PSUM bank); also the x/k chunking
CHUNKS = [(0, 512), (512, 512), (1024, 512), (1536, 512), (2048, 256)]
NCH = len(CHUNKS)
# A-row exp chunks (row col, width): chunk i alternates at1/at2 so each
# chunk's matmuls fit inside the previous exp's window; matmul pieces align
# with the 512-col k chunks
ACHUNKS = [(0, 512), (512, 1024), (1536, 512), (2048, 256)]
NA = len(ACHUNKS)
# o-mm pops allowed after each of the 8 (an, chunk) exp positions per tile
O_COUNTS = [0, 0, 3, 3, 3, 3, 2, 2]

_CACHE = {}


def _build(repeats=1):
    import concourse.bacc as bacc
    import concourse.tile as tile
    from concourse import mybir

    F32 = mybir.dt.float32
    F32R = mybir.dt.float32r
    BF16 = mybir.dt.bfloat16
    AF = mybir.ActivationFunctionType
    ALU = mybir.AluOpType
    AX = mybir.AxisListType

    nc = bacc.Bacc("TRN2", target_bir_lowering=False, debug=False, num_devices=8)

    x1_d = nc.dram_tensor("x1", [C, N], F32R, kind="ExternalInput")
    x2_d = nc.dram_tensor("x2", [C, N], F32R, kind="ExternalInput")
    # consts columns: 0:8 wqT, 8:16 wkT, 16:80 wvT, 80 q1bias, 81 k1bias,
    # 82 q2bias, 83 k2bias (rows 0:8), 84:148 bv1 bcast, 148:212 bv2 bcast,
    # 212 gamma, 213 beta, 214:342 identity (for the residual matmul)
    cst_d = nc.dram_tensor("consts", [P, 342], F32R, kind="ExternalInput")
    out_d = nc.dram_tensor("out", [P, N], F32, kind="ExternalOutput")

    with tile.TileContext(nc) as tc:
        with (
            tc.tile_pool(name="big", bufs=1) as big,
            tc.tile_pool(name="epool", bufs=8) as epool,
            tc.tile_pool(name="small", bufs=4) as small,
            tc.tile_pool(name="psA", bufs=1, space="PSUM") as psA,
            tc.tile_pool(name="psO", bufs=1, space="PSUM") as psO,
        ):
            # ---- early ACT table warm (loads exp tables during DMA wait)
            warm = big.tile([P, 1], F32, name="warm", tag="warm")
            warm2 = big.tile([P, 1], F32, name="warm2", tag="warm2")
            nc.vector.memset(warm, 0.0)
            nc.scalar.activation(out=warm2, in_=warm, func=AF.Exp)

            # ---- constant + input DMAs (x in per-chunk tensors)
            cst = big.tile([P, 342], F32R, name="cst", tag="cst")
            nc.sync.dma_start(out=cst, in_=cst_d.ap())
            xc = []
            for j, (off, w) in enumerate(CHUNKS):
                t_ = big.tile([P, w], F32R, name=f"xc{j}", tag=f"xc{j}")
                nc.sync.dma_start(out=t_[0:C, :], in_=x1_d.ap()[:, off:off + w])
                nc.sync.dma_start(out=t_[C:P, :], in_=x2_d.ap()[:, off:off + w])
                xc.append(t_)

            wq = cst[:, 0:8]
            wk = cst[:, 8:16]
            wv = cst[:, 16:80]
            qkbias = [[cst[:, 80:81].bitcast(F32), cst[:, 81:82].bitcast(F32)],
                      [cst[:, 82:83].bitcast(F32), cst[:, 83:84].bitcast(F32)]]
            bvb = [cst[:, 84:148].bitcast(F32), cst[:, 148:212].bitcast(F32)]
            gamma = cst[:, 212:213].bitcast(F32)
            beta = cst[:, 213:214].bitcast(F32)
            ident = cst[:, 214:342]

            # A-chunk PSUM scratch (separate tensors so PE runs a chunk ahead)
            at2 = psA.tile([P, 1024], F32, name="at2", tag="at2")
            at1 = psA.tile([P, 512], F32, name="at1", tag="at1")
            # o1/o2 accumulator, padded to 5 exact banks (cols 0:N used);
            # also the projection-phase PSUM scratch (free until tile-2 o-mms)
            psum_o_full = psO.tile([P, 2560], F32, name="psum_o", tag="pso")
            psum_o = psum_o_full[:, 0:N]

            # ---- PE HAM warm-up: dummy matmuls during DMA wait
            wz = big.tile([P, 512], BF16, name="wz", tag="wz")
            nc.vector.memset(wz, 0.0)
            for _wi in range(3):
                nc.tensor.matmul(at2[:, 0:512], wz[:, 0:128], wz[:, 0:512])

            # per-(attn, chunk) q/k tensors, rows 0:8 (fp32r full-rate A-mms)
            qc = [[big.tile([CR, w], F32R, name=f"q{an}_{j}", tag=f"q{an}_{j}")
                   for j, (off, w) in enumerate(CHUNKS)] for an in (0, 1)]
            kc = [[big.tile([CR, w], F32R, name=f"k{an}_{j}", tag=f"k{an}_{j}")
                   for j, (off, w) in enumerate(CHUNKS)] for an in (0, 1)]
            # per-row V^T tiles (vt[2t] = attn1/o1, vt[2t+1] = attn2/o2)
            vtb = [big.tile([P, C], F32, name=f"vt{r}", tag=f"vt{r}")
                   for r in range(2 * IT)]
            out_sb = [big.tile([P, w], F32, name=f"osb{j}", tag=f"osb{j}")
                      for j, (off, w) in enumerate(CHUNKS)]

            def emit_compute():
              # ---- q/k projections through psum_o scratch (5 rotating slots)
              ri = 0

              def po_slice(r):
                  b0 = (r % 5) * 512
                  return psum_o_full[:, b0:b0 + 512]

              # both attns' chunk-0 q first, then all k chunks interleaved,
              # then the remaining q chunks (needed from tile 4 on)
              qk_order = [("q", 0, 0)]
              qk_order += [("k", 0, j) for j in range(NCH)]
              qk_order.append(("q", 1, 0))
              qk_order += [("k", 1, j) for j in range(NCH)]
              for j in range(1, NCH):
                  qk_order.append(("q", 0, j))
                  qk_order.append(("q", 1, j))
              for kind, an, j in qk_order:
                  rows = slice(0, C) if an == 0 else slice(C, P)
                  off, w = CHUNKS[j]
                  ws = (wq if kind == "q" else wk)[rows, :]
                  bias = qkbias[an][0 if kind == "q" else 1]
                  dst = (qc if kind == "q" else kc)[an][j]
                  pq = po_slice(ri)
                  ri += 1
                  nc.tensor.matmul(pq[0:CR, :w], ws, xc[j][rows, :])
                  nc.vector.tensor_scalar(
                      out=dst, in0=pq[0:CR, :w],
                      scalar1=bias[0:CR, :], scalar2=None, op0=ALU.add)
              # vt1(0) is needed before the in-loop v pipeline produces it
              pv0 = po_slice(ri)
              ri += 1
              nc.tensor.matmul(pv0[:, 0:C], xc[0][0:C, 0:P], wv[0:C, :])
              nc.vector.tensor_tensor(
                  out=vtb[0], in0=pv0[:, 0:C], in1=bvb[0], op=ALU.add)

              def omm_half(e_rhs, vts_l, prows, st):
                  # one output's 5 chunk matmuls (lhsT stationary across them)
                  return [(psum_o[prows, off:off + w], vts_l,
                           e_rhs[:, off:off + w], st) for (off, w) in CHUNKS]

              def a_slices(ci):
                  if ci == 0:
                      return at1[:, 0:512]
                  if ci == 1:
                      return at2[:, 0:1024]
                  if ci == 2:
                      return at1[:, 0:512]
                  return at2[:, 0:256]

              def emit_ammss(t, an, ci):
                  # A-matmul pieces for chunk ci of row (t, an); pieces align
                  # with the 512-col k chunks
                  base, cw = ACHUNKS[ci]
                  jq = t // 4
                  qlh = qc[an][jq][:, (t % 4) * P:(t % 4 + 1) * P]
                  dst = a_slices(ci)
                  pc = 0
                  while pc < cw:
                      col = base + pc
                      jk = col // 512
                      koff = col % 512
                      w = min(cw - pc, 512 - koff)
                      nc.tensor.matmul(dst[:, pc:pc + w], qlh,
                                       kc[an][jk][:, koff:koff + w])
                      pc += w

              def pop(pending, n):
                  for _ in range(n):
                      if pending:
                          o, l, r_, st_ = pending.pop(0)
                          nc.tensor.matmul(o, l, r_, start=st_, stop=False,
                                           skip_group_check=True)

              # ---- main loop over the 36 (t, an) rows.  A-matmuls for each
              # chunk are emitted one exp-position ahead, so by the time ACT
              # reaches an exp its input matmuls already ran during the
              # previous exp; o-mm pops never sit ahead of A-matmuls.
              rows = [(t, an) for t in range(IT) for an in (0, 1)]
              pending = []
              e_hist = {}
              vtsA = {}
              vtsB = {}
              POPS = [2, 3, 1, 0]
              for r, (t, an) in enumerate(rows):
                  fresh = []
                  if an == 0:
                      if t == 2:
                          # tiles 0/1, deferred: psum_o was projection
                          # scratch until all q/k tensor_scalars retired
                          fresh += omm_half(e_hist[(0, 1)], vtsA[0],
                                            slice(0, C), True)
                          fresh += omm_half(e_hist[(0, 0)], vtsB[0],
                                            slice(C, P), True)
                          fresh += omm_half(e_hist[(1, 1)], vtsA[1],
                                            slice(0, C), False)
                          fresh += omm_half(e_hist[(1, 0)], vtsB[1],
                                            slice(C, P), False)
                      elif t > 2:
                          # o1 of tile t-1 (vtsA(t-1) lands ~1.9us into this
                          # row on DVE -> only enter the pop queue at pos 3)
                          fresh += omm_half(e_hist[(t - 1, 1)], vtsA[t - 1],
                                            slice(0, C), False)
                  if an == 1 and t >= 2:
                      # o2 of tile t (vtsB(t) lands ~1.9us into this row)
                      fresh += omm_half(e_hist[(t, 0)], vtsB[t],
                                        slice(C, P), False)
                  et = epool.tile([P, N], BF16, name=f"e{an}_{t}", tag=f"e{an}")
                  e_hist[(t, an)] = et
                  last_row = r == len(rows) - 1
                  if last_row:
                      sp_last = small.tile([P, NA], F32, name="sp_last",
                                           tag="sp_last", bufs=1)
                  for ci, (base, cw) in enumerate(ACHUNKS):
                      if r == 0 and ci == 0:
                          emit_ammss(t, an, 0)          # bootstrap
                      # one ahead: this position emits the NEXT chunk's mms
                      if ci < NA - 1:
                          emit_ammss(t, an, ci + 1)
                          if ci == NA - 2:
                              # this row's V^T tile through spare at1 columns:
                              # row (t,0) -> vt2(t);  row (t,1) -> vt1(t+1)
                              vr = 2 * t + 1 if an == 0 else 2 * (t + 1)
                              if vr < 2 * IT:
                                  xrows = slice(C, P) if an == 0 else slice(0, C)
                                  vt_t = vr // 2
                                  jx = (vt_t * P) // 512
                                  xoff = (vt_t * P) % 512
                                  nc.tensor.matmul(
                                      at2[:, 256:256 + C],
                                      xc[jx][xrows, xoff:xoff + P],
                                      wv[xrows, :])
                      elif not last_row:
                          emit_ammss(*rows[r + 1], 0)
                      if ci == NA - 1:
                          # fresh o-mms become poppable once their vts landed
                          pending += fresh
                          fresh = []
                      if last_row:
                          spl = sp_last
                          nc.scalar.activation(
                              out=et[:, base:base + cw], in_=a_slices(ci),
                              func=AF.Exp, accum_out=spl[:, ci:ci + 1])
                      else:
                          nc.scalar.activation(
                              out=et[:, base:base + cw], in_=a_slices(ci),
                              func=AF.Exp)
                      pop(pending, POPS[ci])
                      if ci == NA - 2:
                          # V^T tile to SBUF (+bias) during exp2/exp3 so the
                          # next row's at2 matmuls aren't held up by this read
                          vr = 2 * t + 1 if an == 0 else 2 * (t + 1)
                          if vr < 2 * IT:
                              nc.vector.tensor_tensor(
                                  out=vtb[vr], in0=at2[:, 256:256 + C],
                                  in1=bvb[vr % 2], op=ALU.add)
                  sm = small.tile([P, 1], F32, name=f"s{an}_{t}", tag=f"s{an}")
                  if last_row:
                      # sums came from accum_out (ACT is idle at the tail)
                      nc.vector.tensor_reduce(sm, sp_last, axis=AX.X, op=ALU.add)
                  else:
                      # row sums: 3 bf16 tree folds (DVE 2x) + short reduce
                      f1 = small.tile([P, 1152], BF16, name=f"f1_{an}_{t}",
                                      tag="f1", bufs=2)
                      nc.vector.tensor_tensor(
                          out=f1, in0=et[:, 0:1152], in1=et[:, 1152:2304],
                          op=ALU.add)
                      f2 = small.tile([P, 576], BF16, name=f"f2_{an}_{t}",
                                      tag="f2", bufs=2)
                      nc.vector.tensor_tensor(
                          out=f2, in0=f1[:, 0:576], in1=f1[:, 576:1152],
                          op=ALU.add)
                      f3 = small.tile([P, 288], BF16, name=f"f3_{an}_{t}",
                                      tag="f3", bufs=2)
                      nc.vector.tensor_tensor(
                          out=f3, in0=f2[:, 0:288], in1=f2[:, 288:576],
                          op=ALU.add)
                      nc.vector.tensor_reduce(sm, f3, axis=AX.X, op=ALU.add)
                  rr = small.tile([P, 1], F32, name=f"r{an}_{t}", tag=f"r{an}")
                  nc.vector.reciprocal(rr, sm)
                  if an == 0:
                      # o2 weights: vt2(t) * (1/s1) * beta, ready mid-tile
                      vb = small.tile([P, C], BF16, name=f"vB_{t}", tag="vB",
                                      bufs=2)
                      nc.vector.tensor_scalar(
                          out=vb, in0=vtb[2 * t + 1], scalar1=rr,
                          scalar2=beta, op0=ALU.mult, op1=ALU.mult)
                      vtsB[t] = vb
                  else:
                      # o1 weights: vt1(t) * (1/s2) * gamma
                      va = small.tile([P, C], BF16, name=f"vA_{t}", tag="vA",
                                      bufs=2)
                      nc.vector.tensor_scalar(
                          out=va, in0=vtb[2 * t], scalar1=rr,
                          scalar2=gamma, op0=ALU.mult, op1=ALU.mult)
                      vtsA[t] = va

              # ---- tail: leftover o-mms, last tile's o1, residual
              # (identity matmul, stop=True), ScalarE copy + DMA per chunk
              pop(pending, len(pending))
              o1_last = omm_half(e_hist[(IT - 1, 1)], vtsA[IT - 1],
                                 slice(0, C), False)
              for ci, (off, w) in enumerate(CHUNKS):
                  o, l, r_, st_ = o1_last[ci]
                  nc.tensor.matmul(o, l, r_, start=False, stop=False,
                                   skip_group_check=True)
                  nc.tensor.matmul(psum_o[:, off:off + w], ident,
                                   xc[ci][:, 0:w], start=False, stop=True,
                                   skip_group_check=True)
                  nc.scalar.activation(
                      out=out_sb[ci], in_=psum_o[:, off:off + w],
                      func=AF.Copy)
                  nc.sync.dma_start(
                      out=out_d.ap()[:, off:off + w], in_=out_sb[ci])

            if repeats == 1:
                emit_compute()
            else:
                from concourse import mybir as _mb
                with tc.For_i(0, repeats, 1,
                              hint_engines=(_mb.EngineType.PE,
                                            _mb.EngineType.Activation,
                                            _mb.EngineType.DVE)):
                    emit_compute()

    nc.compile()
    return nc


def _get_nc(repeats=1):
    key = f"nc{repeats}"
    if key not in _CACHE:
        _CACHE[key] = _build(repeats)
    return _CACHE[key]


def _make_in_maps(x1, x2, Wqk1, bqk1, Wqk2, bqk2, Wv1, bv1, Wv2, bv2, gamma, beta):
    f = np.float32
    consts = np.zeros((P, 342), dtype=f)
    consts[0:C, 0:8] = np.asarray(Wqk1, f)[0:CR, :].T
    consts[C:P, 0:8] = np.asarray(Wqk2, f)[0:CR, :].T
    consts[0:C, 8:16] = np.asarray(Wqk1, f)[CR:2 * CR, :].T
    consts[C:P, 8:16] = np.asarray(Wqk2, f)[CR:2 * CR, :].T
    consts[0:C, 16:80] = np.asarray(Wv1, f).T
    consts[C:P, 16:80] = np.asarray(Wv2, f).T
    consts[0:CR, 80] = np.asarray(bqk1, f)[0:CR]
    consts[0:CR, 81] = np.asarray(bqk1, f)[CR:2 * CR]
    consts[0:CR, 82] = np.asarray(bqk2, f)[0:CR]
    consts[0:CR, 83] = np.asarray(bqk2, f)[CR:2 * CR]
    consts[:, 84:148] = np.asarray(bv1, f)[None, :]
    consts[:, 148:212] = np.asarray(bv2, f)[None, :]
    consts[:, 212] = np.float32(np.asarray(gamma, f).reshape(-1)[0])
    consts[:, 213] = np.float32(np.asarray(beta, f).reshape(-1)[0])
    consts[:, 214:342] = np.eye(P, dtype=f)

    x1 = np.ascontiguousarray(np.asarray(x1, f).reshape(B, C, N))
    x2 = np.ascontiguousarray(np.asarray(x2, f).reshape(B, C, N))
    return [
        {"x1": np.ascontiguousarray(x1[i]), "x2": np.ascontiguousarray(x2[i]),
         "consts": consts}
        for i in range(B)
    ]


def _run(in_maps, repeats=1, **kwargs):
    from concourse.bass_utils import run_bass_kernel_spmd
    nc = _get_nc(repeats)
    return run_bass_kernel_spmd(nc, in_maps, core_ids=list(range(B)), **kwargs)


def kernel(x1, x2, Wqk1, bqk1, Wqk2, bqk2, Wv1, bv1, Wv2, bv2, gamma, beta):
    in_maps = _make_in_maps(x1, x2, Wqk1, bqk1, Wqk2, bqk2, Wv1, bv1, Wv2, bv2,
                            gamma, beta)
    res = _run(in_maps)
    o1 = np.empty((B, C, H, W), dtype=np.float32)
    o2 = np.empty((B, C, H, W), dtype=np.float32)
    for i in range(B):
        full = res.results[i]["out"]
        o1[i] = full[0:C, :].reshape(C, H, W)
        o2[i] = full[C:P, :].reshape(C, H, W)
    return o1, o2
